# revision 17
# baseline (speedup 1.0000x reference)
"""Trainium2 Bass kernel for the DecoderCRF loss (B=64, S=512, D=512, T=12).

Math
----
reference loss = sum_b [ logZ_b - gold_b ] with feats = x @ W.T + b.

For the transitions matrix this problem ships (row START == -1e4, col
STOP == -1e4, everything else 0) and an all-ones mask, the forward
recursion collapses exactly (verified vs a float64 port of the reference):

    logZ_b  = sum_t log( sum_{j=0..9} exp(feats[b,t,j]) )
    gold_b  = sum_t feats[b,t,tags[b,t]]

feats entries are ~N(0,1) (range +-7), so exp() needs no max-shift in f32.

Layout strategy (v2)
--------------------
f32 matmuls on the PE run 2-pass (fp32_mode=LOW_HIGH) and PE-side
transposes of x dominated v1 (PE 83us busy).  bf16 is numerically ample
here (loss abs err ~0.4 on 9.5e4), so the host ships x already
TRANSPOSED and CAST to bf16 — that removes every PE transpose, every
PSUM->SBUF copy, and halves the DMA bytes.  Data-parallel over batch:
core c handles batch elements 8c..8c+7.

Per core (xT [512 d, 4096 s] bf16, processed in 4 s-panels of 1024):
  1. featsT panel = sum_dc WtT_dc @ xT[dc, panel]    -> PSUM [10, 1024] f32
  2. E = exp(featsT + bias)  (ScalarE, bf16 out)
  3. per 512-half: ones-selector matmul accumulates col-sums of E into
     row g of a persistent PSUM [8, 512]  (partition reduction on PE)
  4. gold: DVE multiplies featsT PSUM by host-built onehotT panel
Tail: ln(colsums) + row-sum -> [8,1]; gold reduce -> [10,1]; DMA out.
Host sums partials (and the sum_t bias[tag] term) in f64.

Non-conforming inputs (different transitions pattern / mask / tag range)
fall back to a faithful numpy port of the reference.
"""

from contextlib import ExitStack

import numpy as np

N_CORES = 8
B, S, D = 64, 512, 512
T = 12
NT = 10          # tags that can actually appear / participate in the LSE
START, STOP = 10, 11
NEG = -10000.0
BS = B // N_CORES          # batch elements per core
R = BS * S                 # s-rows per core (4096)
N_GROUPS = 8               # 512-row groups per core (one batch element each)
GROUP = R // N_GROUPS      # 512
N_PANELS = 4               # s-panels per core
PANEL = R // N_PANELS      # 1024

_NC_CACHE = None


def _build_nc():
    import concourse.bacc as bacc
    import concourse.mybir as mybir
    import concourse.tile as tile

    f32 = mybir.dt.float32
    bf16 = mybir.dt.bfloat16
    nc = bacc.Bacc("TRN2", target_bir_lowering=False, num_swdge_queues=4)

    # panel-major layout: per panel, each SBUF partition reads one
    # contiguous 8KB run -> few fat DMA descriptors, fast Q7 emission
    xt_d = nc.dram_tensor("xt", [N_PANELS, 128, 4, PANEL], bf16, kind="ExternalInput")
    oht_d = nc.dram_tensor("oht", [NT, R], f32, kind="ExternalInput")
    wt_d = nc.dram_tensor("wt", [D, NT], bf16, kind="ExternalInput")
    gsel_d = nc.dram_tensor("gsel", [NT, 8 * N_GROUPS], bf16, kind="ExternalInput")
    b10_d = nc.dram_tensor("b10", [NT, 1], f32, kind="ExternalInput")
    out_c_d = nc.dram_tensor("out_c", [N_GROUPS, GROUP], f32, kind="ExternalOutput")
    out_g_d = nc.dram_tensor("out_g", [1, GROUP], f32, kind="ExternalOutput")

    wt_r = wt_d.rearrange("(c p) m -> p c m", p=128)   # [128, 4, 10]

    with tile.TileContext(nc) as tc, ExitStack() as ctx:
        consts = ctx.enter_context(tc.tile_pool(name="consts", bufs=1))
        xtp = ctx.enter_context(tc.tile_pool(name="xtp", bufs=4))
        epool = ctx.enter_context(tc.tile_pool(name="epool", bufs=2))
        fin = ctx.enter_context(tc.tile_pool(name="fin", bufs=1))
        pf = ctx.enter_context(tc.tile_pool(name="pf", bufs=2, space="PSUM"))
        pacc = ctx.enter_context(tc.tile_pool(name="pacc", bufs=1, space="PSUM"))

        wt_sb = consts.tile([128, 4, NT], bf16)
        nc.sync.dma_start(out=wt_sb, in_=wt_r)
        oht_sb = consts.tile([NT, R], f32)
        nc.gpsimd.dma_start(out=oht_sb, in_=oht_d[:, :])
        gsel_sb = consts.tile([NT, 8 * N_GROUPS], bf16)
        nc.sync.dma_start(out=gsel_sb, in_=gsel_d[:, :])
        b10_sb = consts.tile([NT, 1], f32)
        nc.sync.dma_start(out=b10_sb, in_=b10_d[:, :])

        psum_c = pacc.tile([N_GROUPS, GROUP], f32, tag="csum")
        psum_gold = pacc.tile([1, GROUP], f32, tag="gsum")
        ones10 = consts.tile([NT, 1], bf16, tag="ones10")
        nc.vector.memset(ones10, 1.0)

        warm = consts.tile([128, GROUP], bf16, tag="warm")
        nc.vector.memset(warm, 0.0)
        psum_w = pacc.tile([128, GROUP], f32, tag="warm")
        for _ in range(12):
            nc.tensor.matmul(psum_w, lhsT=warm[:, 0:128], rhs=warm,
                             start=True, stop=True)

        # kick all panel loads first so the SWDGE stream is never compute-gated
        xt_tiles = []
        for p in range(N_PANELS):
            # SWDGE: descriptor swizzle spreads the 1MB panel across all
            # 16 SDMA engines (HWDGE put everything on one engine)
            xt_sb = xtp.tile([128, 4, PANEL], bf16)
            nc.gpsimd.dma_start(out=xt_sb, in_=xt_d[p])
            xt_tiles.append(xt_sb)

        for p in range(N_PANELS):
            xt_sb = xt_tiles[p]
            psum_f = pf.tile([NT, PANEL], f32)
            for dc in range(4):
                for h in range(2):      # matmul out must stay within one PSUM bank
                    nc.tensor.matmul(
                        psum_f[:, h * GROUP : (h + 1) * GROUP],
                        lhsT=wt_sb[:, dc, :],
                        rhs=xt_sb[:, dc, h * GROUP : (h + 1) * GROUP],
                        start=(dc == 0),
                        stop=(dc == 3),
                    )

            e_sb = epool.tile([NT, PANEL], bf16)
            for h in range(2):          # two 512-col groups per panel
                g = 2 * p + h
                nc.scalar.activation(
                    e_sb[:, h * GROUP : (h + 1) * GROUP],
                    psum_f[:, h * GROUP : (h + 1) * GROUP],
                    mybir.ActivationFunctionType.Exp,
                    bias=b10_sb[:, :],
                )
                nc.tensor.matmul(
                    psum_c,
                    lhsT=gsel_sb[:, 8 * g : 8 * (g + 1)],
                    rhs=e_sb[:, h * GROUP : (h + 1) * GROUP],
                    start=(g == 0),
                    stop=(g == 2 * N_PANELS - 1),
                )

            gw = xtp.tile([NT, PANEL], bf16, tag="gw")
            nc.vector.tensor_mul(gw, psum_f, oht_sb[:, p * PANEL : (p + 1) * PANEL])
            for h in range(2):   # PE sums gw over tags AND s-halves into [1, 512]
                nc.tensor.matmul(
                    psum_gold,
                    lhsT=ones10,
                    rhs=gw[:, h * GROUP : (h + 1) * GROUP],
                    start=(p == 0 and h == 0),
                    stop=(p == N_PANELS - 1 and h == 1),
                )

        csum_sb = fin.tile([N_GROUPS, GROUP], f32)
        nc.scalar.copy(out=csum_sb, in_=psum_c)
        nc.sync.dma_start(out=out_c_d[:, :], in_=csum_sb)
        gsum_sb = fin.tile([1, GROUP], f32)
        nc.vector.tensor_copy(out=gsum_sb, in_=psum_gold)
        nc.sync.dma_start(out=out_g_d[:, :], in_=gsum_sb)

    nc.compile()
    return nc


def _get_nc():
    global _NC_CACHE
    if _NC_CACHE is None:
        _NC_CACHE = _build_nc()
    return _NC_CACHE


def _fast_path_ok(transitions, tags, mask):
    if transitions.shape != (T, T) or tags.min() < 0 or tags.max() >= NT:
        return False
    if not np.all(mask == 1):
        return False
    t2 = np.asarray(transitions, np.float64).copy()
    if not (np.all(t2[START, :] == NEG) and np.all(t2[:, STOP] == NEG)):
        return False
    t2[START, :] = 0.0
    t2[:, STOP] = 0.0
    return bool(np.all(t2 == 0.0))


def _reference_numpy(input_var, W, b, transitions, tags, mask):
    """Faithful float64 port of the reference (fallback only)."""
    x = np.asarray(input_var, np.float64)
    Wf = np.asarray(W, np.float64)
    bf = np.asarray(b, np.float64)
    tr = np.asarray(transitions, np.float64)
    mf = np.asarray(mask, np.float64)
    Bn, Sn, Dn = x.shape
    feats = (x.reshape(-1, Dn) @ Wf.T + bf).reshape(Bn, Sn, -1)
    fv = np.full((Bn, T), NEG)
    fv[:, START] = 0.0
    for t in range(Sn):
        tv = fv[:, None, :] + tr[None] + feats[:, t][:, :, None]
        m = tv.max(axis=2)
        new = m + np.log(np.exp(tv - m[:, :, None]).sum(axis=2))
        fv = new * mf[:, t : t + 1] + fv * (1 - mf[:, t : t + 1])
    fin = fv + tr[STOP][None]
    mm = fin.max(axis=1)
    alpha = mm + np.log(np.exp(fin - mm[:, None]).sum(axis=1))
    score0 = tr[tags[:, 0], START]
    emit = np.take_along_axis(feats[:, :-1], tags[:, :-1, None], axis=2)[..., 0]
    emit_sum = (emit * mf[:, :-1]).sum(axis=1)
    trs = tr[tags[:, 1:], tags[:, :-1]]
    trans_sum = (trs * mf[:, 1:]).sum(axis=1)
    last_idx = np.asarray(mask).sum(axis=1).astype(np.int64) - 1
    last_tags = np.take_along_axis(tags, last_idx[:, None], axis=1)[:, 0]
    last_emit = np.take_along_axis(feats[:, -1], last_tags[:, None], axis=1)[:, 0]
    gold = score0 + emit_sum + trans_sum + tr[STOP, last_tags] + last_emit * mf[:, -1]
    return np.float32((alpha - gold).sum())


def _make_in_maps(input_var, W, b, tags):
    import ml_dtypes

    bf16 = ml_dtypes.bfloat16
    wt = np.ascontiguousarray(W[:NT].T).astype(bf16)                # [512, 10]
    b10 = np.ascontiguousarray(b[:NT].reshape(NT, 1), np.float32)
    gsel = np.zeros((NT, 8 * N_GROUPS), np.float32)
    for g in range(N_GROUPS):
        gsel[:, 8 * g + g] = 1.0
    gsel = gsel.astype(bf16)

    xbf = input_var.reshape(B * S, D).astype(bf16)                  # one big cast
    onehot = np.zeros((B * S, NT), np.float32)
    onehot[np.arange(B * S), tags.reshape(-1)] = 1.0

    in_maps = []
    for c in range(N_CORES):
        xt = np.ascontiguousarray(xbf[c * R : (c + 1) * R].T)       # [512, 4096] bf16
        # [dc, p, panel, s] -> [panel, p, dc, s] so each partition's panel
        # data is one contiguous 8KB run in DRAM
        xtp = np.ascontiguousarray(
            xt.reshape(4, 128, N_PANELS, PANEL).transpose(2, 1, 0, 3)
        )
        oht = np.ascontiguousarray(onehot[c * R : (c + 1) * R].T)   # [10, 4096] f32
        in_maps.append(
            {"xt": xtp, "oht": oht, "wt": wt, "gsel": gsel, "b10": b10}
        )
    return in_maps


def kernel(input_var, W, b, transitions, tags, mask):
    from concourse.bass_utils import run_bass_kernel_spmd

    input_var = np.asarray(input_var)
    W = np.asarray(W)
    b = np.asarray(b)
    transitions = np.asarray(transitions)
    tags = np.asarray(tags)
    mask = np.asarray(mask)

    if not _fast_path_ok(transitions, tags, mask):
        return _reference_numpy(input_var, W, b, transitions, tags, mask)

    nc = _get_nc()
    in_maps = _make_in_maps(input_var, W, b, tags)
    res = run_bass_kernel_spmd(nc, in_maps, list(range(N_CORES)))

    total = np.float64(0.0)
    for c in range(N_CORES):
        csum = np.asarray(res.results[c]["out_c"], np.float64)   # [8, 512]
        gsum = np.asarray(res.results[c]["out_g"], np.float64)   # [1, 512]
        total += np.log(csum).sum() - gsum.sum()
    total -= np.asarray(b, np.float64)[tags].sum()   # gold bias term, host-side
    return np.float32(total)
